# revision 12
# baseline (speedup 1.0000x reference)
"""Masked-BCE mean loss kernel for Trainium2, data-parallel over 8 NeuronCores.

Math (targets t are exactly 0.0/1.0, C=2 label columns):
    bce(x, t) = softplus(x) - x*t = softplus(y),  y = (1-2t)*x = w*x
    row mask  = 1[t0 + t1 > 0]
    answer    = sum(mask * (bce0 + bce1)) / (B*C)

Work is split across chunks in two modes, balancing DVE against ACT with the
tensor engine absorbing the remainder:

  PAIR chunks (DVE-heavy, ACT-light):
    softplus(y0)+softplus(y1) = ln((1+E0)(1+E1)) = ln(1+r), r = E0+E1+E0*E1.
    The pair mask M' multiplies into r before the Ln (bias=1.0), so masked
    pairs give ln(1)=0 and the Ln's accum_out IS the masked partial sum.
    Ln runs on HALF the elements.

  FULL chunks (ACT-heavy, DVE-light, PE does the masking):
    S = ln(E+1) over all elements; PE accumulates the generalized diagonal
    of M'[128-window]^T x [S0-win | S1-win] (FD=256) into PSUM -- the
    diagonal stripes hold sum(mask * (sp0+sp1)).

Common per-chunk DVE (all unit-stride bf16 -> 2x mode; layout [c0 | c1]
half-blocks per chunk):
    Y  = W * X          (exact: w is +-1)
    V  = min(W0, W1)    (+1 on all-zero-target rows, -1 otherwise)
    M' = -0.5*V + 0.5   (tensor_scalar 2-op, 4x; exact {0,1})

Engines run their queues IN ORDER, so emission is software-pipelined one
chunk deep (a stalled consumer never blocks the next chunk's producer), and
all input DMAs are issued up front on two queues.

Host: answer = (sum(scol) + sum of PSUM diag stripes) / (B*C) in f64.
"""

import sys

import numpy as np

for _p in ("/opt/trn_rl_repo",):
    if _p not in sys.path:
        sys.path.insert(0, _p)

import concourse.tile as tile  # noqa: E402
from concourse import bacc, mybir  # noqa: E402
from concourse.bass_utils import run_bass_kernel_spmd  # noqa: E402

N_CORES = 8
B = 8388608
C = 2
PAIRS = B // N_CORES  # 1048576 pairs per core
P = 128
COLS = 2 * PAIRS // P  # 16384 total columns per core
HCOLS = COLS // 2  # 8192 pair-columns per core

# chunk schedule: (h pair-cols, mode); FULL chunks first so the PE stripe
# matmuls drain while the PAIR chunks still run
CHUNKS = [
    (512, "full"),
    (1024, "full"),
    (1024, "full"),
    (1792, "pair"),
    (1792, "pair"),
    (2048, "pair"),
]
assert sum(h for h, _ in CHUNKS) == HCOLS
N_CHUNKS = len(CHUNKS)
N_PAIR = sum(1 for _, m in CHUNKS if m == "pair")
TOTAL_MM = sum(h // 128 for h, m in CHUNKS if m == "full")

dt = mybir.dt
AF = mybir.ActivationFunctionType
ALU = mybir.AluOpType

_CACHE: dict[str, object] = {}


def _patch_act_tables():
    """Pin Exp and Ln to the single covering table set (one ACT_TABLE_LOAD)."""
    if _CACHE.get("act_patched"):
        return
    import concourse.hw_specs as hw_specs

    orig = hw_specs.get_activation_tables

    def patched(module_arch):
        tabs = orig(module_arch)
        out = {}
        for name, funcs in tabs.items():
            if name == "natural_log_exp_and_others":
                out[name] = set(funcs)
            else:
                out[name] = set(funcs) - {AF.Exp, AF.Ln}
        return out

    bacc.get_activation_tables = patched
    _CACHE["act_patched"] = True


def _build_nc():
    _patch_act_tables()
    nc = bacc.Bacc(
        "TRN2", target_bir_lowering=False, debug=False, num_devices=N_CORES
    )
    x_d = nc.dram_tensor("x", [P, COLS], dt.bfloat16, kind="ExternalInput").ap()
    w_d = nc.dram_tensor("w", [P, COLS], dt.bfloat16, kind="ExternalInput").ap()
    scol_d = nc.dram_tensor(
        "scol", [P, N_CHUNKS], dt.float32, kind="ExternalOutput"
    ).ap()
    acc_d = nc.dram_tensor("acc", [P, 256], dt.float32, kind="ExternalOutput").ap()

    with tile.TileContext(nc) as tc:
        with (
            tc.tile_pool(name="io", bufs=N_CHUNKS) as io_pool,
            tc.tile_pool(name="work", bufs=2) as work_pool,
            tc.tile_pool(name="ps", bufs=1, space="PSUM") as psum_pool,
            tc.tile_pool(name="outp", bufs=1) as out_pool,
        ):
            # tiny dummy Exp up front hoists the ~2.7us ACT_TABLE_LOAD off
            # the critical path (overlaps the first DMAs)
            warm = out_pool.tile([P, 8], dt.float32)
            nc.vector.memset(warm[:], 0.0)
            nc.scalar.activation(warm[:], warm[:], AF.Exp)

            scol = out_pool.tile([P, N_CHUNKS], dt.float32)
            nc.vector.memset(scol[:], 0.0)
            acc = psum_pool.tile([P, 256], dt.float32)

            # all input DMAs up front; X on the SP queue, W on the GPSIMD
            # queue so dispatch overlaps; each chunk owns its tiles
            X, W = [], []
            col0 = 0
            for ci, (h, _) in enumerate(CHUNKS):
                f = 2 * h
                Xc = io_pool.tile([P, f], dt.bfloat16, tag="X", name=f"X{ci}")
                nc.sync.dma_start(Xc[:], x_d[:, col0 : col0 + f])
                Wc = io_pool.tile([P, f], dt.bfloat16, tag="W", name=f"W{ci}")
                nc.gpsimd.dma_start(Wc[:], w_d[:, col0 : col0 + f])
                X.append(Xc)
                W.append(Wc)
                col0 += f

            E = [None] * N_CHUNKS
            Mp = [None] * N_CHUNKS
            mm_state = [0]

            def stage_B(ci):  # input-side DVE + exp
                h, _ = CHUNKS[ci]
                f = 2 * h
                Y = work_pool.tile([P, f], dt.bfloat16, tag="Y")
                nc.vector.tensor_tensor(Y[:], W[ci][:], X[ci][:], ALU.mult)
                E[ci] = work_pool.tile(
                    [P, f], dt.bfloat16, tag="E", name=f"E{ci}"
                )
                nc.scalar.activation(E[ci][:], Y[:], AF.Exp)
                V = work_pool.tile([P, h], dt.bfloat16, tag="V")
                nc.vector.tensor_tensor(
                    V[:], W[ci][:, :h], W[ci][:, h:f], ALU.min
                )
                Mp[ci] = work_pool.tile(
                    [P, h], dt.bfloat16, tag="Mp", name=f"Mp{ci}"
                )
                nc.vector.tensor_scalar(
                    Mp[ci][:], V[:], -0.5, 0.5, ALU.mult, ALU.add
                )

            def stage_C(ci):
                h, mode = CHUNKS[ci]
                f = 2 * h
                Ec = E[ci]
                if mode == "pair":
                    P2 = work_pool.tile([P, h], dt.bfloat16, tag="P2")
                    nc.vector.scalar_tensor_tensor(
                        P2[:], Ec[:, :h], 1.0, Ec[:, h:f], ALU.add, ALU.mult
                    )
                    r = work_pool.tile([P, h], dt.bfloat16, tag="r")
                    nc.vector.tensor_tensor(r[:], P2[:], Ec[:, :h], ALU.add)
                    rm = work_pool.tile([P, h], dt.bfloat16, tag="rm")
                    nc.vector.tensor_tensor(rm[:], r[:], Mp[ci][:], ALU.mult)
                    L = work_pool.tile([P, h], dt.bfloat16, tag="L")
                    nc.scalar.activation(
                        L[:], rm[:], AF.Ln, bias=1.0,
                        accum_out=scol[:, ci : ci + 1],
                    )
                else:
                    S = work_pool.tile([P, f], dt.bfloat16, tag="S")
                    nc.scalar.activation(S[:], Ec[:], AF.Ln, bias=1.0)
                    Sv = S[:].rearrange("p (b h) -> p b h", b=2)
                    for c in range(0, h, 128):
                        nc.tensor.matmul(
                            acc[:],
                            lhsT=Mp[ci][:, c : c + 128],
                            rhs=Sv[:, :, c : c + 128],
                            start=(mm_state[0] == 0),
                            stop=(mm_state[0] == TOTAL_MM - 1),
                        )
                        mm_state[0] += 1

            # software-pipelined emission, one chunk deep
            stage_B(0)
            for ci in range(1, N_CHUNKS):
                stage_B(ci)
                stage_C(ci - 1)
            stage_C(N_CHUNKS - 1)

            out_acc = out_pool.tile([P, 256], dt.float32)
            nc.vector.tensor_copy(out_acc[:], acc[:])
            nc.sync.dma_start(acc_d[:], out_acc[:])
            nc.sync.dma_start(scol_d[:], scol[:])

    nc.compile()
    return nc


def _get_nc():
    if "nc" not in _CACHE:
        _CACHE["nc"] = _build_nc()
    return _CACHE["nc"]


def _reduce_outputs(
    scols: list[np.ndarray], accs: list[np.ndarray]
) -> np.ndarray:
    j = np.arange(P)
    total = 0.0
    for sc, ac in zip(scols, accs):
        total += sc.astype(np.float64).sum()  # pair chunks: masked sums
        a64 = ac.astype(np.float64)
        total += a64[j, j].sum() + a64[j, 128 + j].sum()  # full chunks
    return np.asarray(total / (B * C), dtype=np.float32)


def make_in_maps(inputs: np.ndarray, targets: np.ndarray) -> list[dict]:
    import ml_dtypes

    # Layout per core: pairs deinterleaved into [col0 | col1] blocks per chunk
    # so all device DVE ops are unit-stride (2x mode).  w = 1 - 2t (+-1, exact
    # in bf16) multiplies straight into x on device.
    x = np.ascontiguousarray(inputs, dtype=np.float32).reshape(
        N_CORES, PAIRS, C
    )
    w = 1.0 - 2.0 * np.ascontiguousarray(targets, dtype=np.float32).reshape(
        N_CORES, PAIRS, C
    )
    xp = x.transpose(0, 2, 1).reshape(N_CORES, C, P, HCOLS)
    wp = w.transpose(0, 2, 1).reshape(N_CORES, C, P, HCOLS)

    xs = np.empty((N_CORES, P, COLS), dtype=ml_dtypes.bfloat16)
    ws = np.empty((N_CORES, P, COLS), dtype=ml_dtypes.bfloat16)
    col0 = 0
    off = 0
    for h, _ in CHUNKS:
        xs[:, :, col0 : col0 + h] = xp[:, 0, :, off : off + h]
        xs[:, :, col0 + h : col0 + 2 * h] = xp[:, 1, :, off : off + h]
        ws[:, :, col0 : col0 + h] = wp[:, 0, :, off : off + h]
        ws[:, :, col0 + h : col0 + 2 * h] = wp[:, 1, :, off : off + h]
        col0 += 2 * h
        off += h
    return [{"x": xs[c], "w": ws[c]} for c in range(N_CORES)]


def kernel(inputs: np.ndarray, targets: np.ndarray) -> np.ndarray:
    nc = _get_nc()
    in_maps = make_in_maps(inputs, targets)
    res = run_bass_kernel_spmd(nc, in_maps, list(range(N_CORES)))
    scols = [res.results[c]["scol"] for c in range(N_CORES)]
    accs = [res.results[c]["acc"] for c in range(N_CORES)]
    return _reduce_outputs(scols, accs)


# revision 13
# speedup vs baseline: 1.1724x; 1.1724x over previous
"""Masked-BCE mean loss kernel for Trainium2, data-parallel over 8 NeuronCores.

Math (targets t are exactly 0.0/1.0, C=2 label columns):
    bce(x, t) = softplus(x) - x*t = softplus(y),  y = (1-2t)*x = w*x
    row mask  = 1[t0 + t1 > 0]
    answer    = sum(mask * (bce0 + bce1)) / (B*C)

Work is split across chunks in two modes, balancing DVE against ACT with the
tensor engine absorbing the remainder:

  PAIR chunks (DVE-heavy, ACT-light):
    softplus(y0)+softplus(y1) = ln((1+E0)(1+E1)) = ln(1+r), r = E0+E1+E0*E1.
    The pair mask M' multiplies into r before the Ln (bias=1.0), so masked
    pairs give ln(1)=0 and the Ln's accum_out IS the masked partial sum.
    Ln runs on HALF the elements.

  FULL chunks (ACT-heavy, DVE-light, PE does the masking):
    S = ln(E+1) over all elements; PE accumulates the generalized diagonal
    of M'[128-window]^T x [S0-win | S1-win] (FD=256) into PSUM -- the
    diagonal stripes hold sum(mask * (sp0+sp1)).

Common per-chunk DVE (all unit-stride bf16 -> 2x mode; layout [c0 | c1]
half-blocks per chunk):
    Y  = W * X          (exact: w is +-1)
    V  = min(W0, W1)    (+1 on all-zero-target rows, -1 otherwise)
    M' = -0.5*V + 0.5   (tensor_scalar 2-op, 4x; exact {0,1})

DMA discipline (every dma_start pays ~2us fixed and all transfers FIFO on
one SDMA ring set): X and W ship interleaved in ONE dram tensor, one
dma_start per chunk (large contiguous descriptors), all via nc.sync (HWDGE;
SWDGE would starve behind DVE perf-mode ops).  Outputs merge into one
tensor/one DMA.  Engines run their queues IN ORDER, so emission is
software-pipelined one chunk deep.

Host: answer = (scol part + PSUM diag stripes part) / (B*C) in f64.
"""

import sys

import numpy as np

for _p in ("/opt/trn_rl_repo",):
    if _p not in sys.path:
        sys.path.insert(0, _p)

import concourse.tile as tile  # noqa: E402
from concourse import bacc, mybir  # noqa: E402
from concourse.bass_utils import run_bass_kernel_spmd  # noqa: E402

N_CORES = 8
B = 8388608
C = 2
PAIRS = B // N_CORES  # 1048576 pairs per core
P = 128
COLS = 2 * PAIRS // P  # 16384 data columns per core
HCOLS = COLS // 2  # 8192 pair-columns per core

# chunk schedule: (h pair-cols, mode); FULL chunks first so the PE stripe
# matmuls drain while the PAIR chunks still run
CHUNKS = [
    (512, "full"),
    (1024, "full"),
    (1024, "full"),
    (1792, "pair"),
    (1792, "pair"),
    (2048, "pair"),
]
assert sum(h for h, _ in CHUNKS) == HCOLS
N_CHUNKS = len(CHUNKS)
TOTAL_MM = sum(h // 128 for h, m in CHUNKS if m == "full")

dt = mybir.dt
AF = mybir.ActivationFunctionType
ALU = mybir.AluOpType

_CACHE: dict[str, object] = {}


def _patch_act_tables():
    """Pin Exp and Ln to the single covering table set (one ACT_TABLE_LOAD)."""
    if _CACHE.get("act_patched"):
        return
    import concourse.hw_specs as hw_specs

    orig = hw_specs.get_activation_tables

    def patched(module_arch):
        tabs = orig(module_arch)
        out = {}
        for name, funcs in tabs.items():
            if name == "natural_log_exp_and_others":
                out[name] = set(funcs)
            else:
                out[name] = set(funcs) - {AF.Exp, AF.Ln}
        return out

    bacc.get_activation_tables = patched
    _CACHE["act_patched"] = True


def _build_nc():
    _patch_act_tables()
    nc = bacc.Bacc(
        "TRN2", target_bir_lowering=False, debug=False, num_devices=N_CORES
    )
    # X and W interleaved chunk-major: chunk ci occupies 4*h cols as
    # [X0h | X1h | W0h | W1h]
    xw_d = nc.dram_tensor(
        "xw", [P, 2 * COLS], dt.bfloat16, kind="ExternalInput"
    ).ap()
    # one output tensor: cols 0..255 = PE stripe acc, 256.. = ln accums
    res_d = nc.dram_tensor(
        "res", [P, 256 + N_CHUNKS], dt.float32, kind="ExternalOutput"
    ).ap()

    with tile.TileContext(nc) as tc:
        with (
            tc.tile_pool(name="io", bufs=N_CHUNKS) as io_pool,
            tc.tile_pool(name="work", bufs=2) as work_pool,
            tc.tile_pool(name="ps", bufs=1, space="PSUM") as psum_pool,
            tc.tile_pool(name="outp", bufs=1) as out_pool,
        ):
            # tiny dummy Exp up front hoists the ~2.7us ACT_TABLE_LOAD off
            # the critical path (overlaps the first DMAs)
            warm = out_pool.tile([P, 8], dt.float32)
            nc.vector.memset(warm[:], 0.0)
            nc.scalar.activation(warm[:], warm[:], AF.Exp)

            res = out_pool.tile([P, 256 + N_CHUNKS], dt.float32)
            nc.vector.memset(res[:, 256:], 0.0)
            acc = psum_pool.tile([P, 256], dt.float32)

            # one input DMA per chunk; each chunk owns its tile
            XW = []
            col0 = 0
            for ci, (h, _) in enumerate(CHUNKS):
                T = io_pool.tile([P, 4 * h], dt.bfloat16, tag="XW",
                                 name=f"XW{ci}")
                nc.sync.dma_start(T[:], xw_d[:, col0 : col0 + 4 * h])
                XW.append(T)
                col0 += 4 * h

            E = [None] * N_CHUNKS
            Mp = [None] * N_CHUNKS
            mm_state = [0]

            def stage_B(ci):  # input-side DVE + exp
                h, _ = CHUNKS[ci]
                f = 2 * h
                Xc = XW[ci][:, :f]
                Wc = XW[ci][:, f : 2 * f]
                Y = work_pool.tile([P, f], dt.bfloat16, tag="Y")
                nc.vector.tensor_tensor(Y[:], Wc, Xc, ALU.mult)
                E[ci] = work_pool.tile(
                    [P, f], dt.bfloat16, tag="E", name=f"E{ci}"
                )
                nc.scalar.activation(E[ci][:], Y[:], AF.Exp)
                V = work_pool.tile([P, h], dt.bfloat16, tag="V")
                nc.vector.tensor_tensor(
                    V[:], XW[ci][:, f : f + h], XW[ci][:, f + h : 2 * f],
                    ALU.min,
                )
                Mp[ci] = work_pool.tile(
                    [P, h], dt.bfloat16, tag="Mp", name=f"Mp{ci}"
                )
                nc.vector.tensor_scalar(
                    Mp[ci][:], V[:], -0.5, 0.5, ALU.mult, ALU.add
                )

            def stage_C(ci):
                h, mode = CHUNKS[ci]
                f = 2 * h
                Ec = E[ci]
                if mode == "pair":
                    P2 = work_pool.tile([P, h], dt.bfloat16, tag="P2")
                    nc.vector.scalar_tensor_tensor(
                        P2[:], Ec[:, :h], 1.0, Ec[:, h:f], ALU.add, ALU.mult
                    )
                    r = work_pool.tile([P, h], dt.bfloat16, tag="r")
                    nc.vector.tensor_tensor(r[:], P2[:], Ec[:, :h], ALU.add)
                    rm = work_pool.tile([P, h], dt.bfloat16, tag="rm")
                    nc.vector.tensor_tensor(rm[:], r[:], Mp[ci][:], ALU.mult)
                    L = work_pool.tile([P, h], dt.bfloat16, tag="L")
                    nc.scalar.activation(
                        L[:], rm[:], AF.Ln, bias=1.0,
                        accum_out=res[:, 256 + ci : 257 + ci],
                    )
                else:
                    S = work_pool.tile([P, f], dt.bfloat16, tag="S")
                    nc.scalar.activation(S[:], Ec[:], AF.Ln, bias=1.0)
                    Sv = S[:].rearrange("p (b h) -> p b h", b=2)
                    for c in range(0, h, 128):
                        nc.tensor.matmul(
                            acc[:],
                            lhsT=Mp[ci][:, c : c + 128],
                            rhs=Sv[:, :, c : c + 128],
                            start=(mm_state[0] == 0),
                            stop=(mm_state[0] == TOTAL_MM - 1),
                        )
                        mm_state[0] += 1

            # software-pipelined emission, one chunk deep
            stage_B(0)
            for ci in range(1, N_CHUNKS):
                stage_B(ci)
                stage_C(ci - 1)
            stage_C(N_CHUNKS - 1)

            nc.vector.tensor_copy(res[:, :256], acc[:])
            nc.sync.dma_start(res_d[:], res[:])

    nc.compile()
    return nc


def _get_nc():
    if "nc" not in _CACHE:
        _CACHE["nc"] = _build_nc()
    return _CACHE["nc"]


def _reduce_outputs(results: list[np.ndarray]) -> np.ndarray:
    j = np.arange(P)
    total = 0.0
    for re_ in results:
        a64 = re_.astype(np.float64)
        total += a64[:, 256:].sum()  # pair chunks: ln accums
        total += a64[j, j].sum() + a64[j, 128 + j].sum()  # full chunks
    return np.asarray(total / (B * C), dtype=np.float32)


def make_in_maps(inputs: np.ndarray, targets: np.ndarray) -> list[dict]:
    import ml_dtypes

    # Per core, chunk-major interleave [X0h | X1h | W0h | W1h] so one
    # dma_start per chunk carries both tensors; halves are unit-stride
    # blocks so every DVE op runs in 2x mode.  w = 1 - 2t (+-1, exact in
    # bf16) multiplies straight into x on device.
    x = np.ascontiguousarray(inputs, dtype=np.float32).reshape(
        N_CORES, PAIRS, C
    )
    w = 1.0 - 2.0 * np.ascontiguousarray(targets, dtype=np.float32).reshape(
        N_CORES, PAIRS, C
    )
    xp = x.transpose(0, 2, 1).reshape(N_CORES, C, P, HCOLS)
    wp = w.transpose(0, 2, 1).reshape(N_CORES, C, P, HCOLS)

    xw = np.empty((N_CORES, P, 2 * COLS), dtype=ml_dtypes.bfloat16)
    col0 = 0
    off = 0
    for h, _ in CHUNKS:
        xw[:, :, col0 : col0 + h] = xp[:, 0, :, off : off + h]
        xw[:, :, col0 + h : col0 + 2 * h] = xp[:, 1, :, off : off + h]
        xw[:, :, col0 + 2 * h : col0 + 3 * h] = wp[:, 0, :, off : off + h]
        xw[:, :, col0 + 3 * h : col0 + 4 * h] = wp[:, 1, :, off : off + h]
        col0 += 4 * h
        off += h
    return [{"xw": xw[c]} for c in range(N_CORES)]


def kernel(inputs: np.ndarray, targets: np.ndarray) -> np.ndarray:
    nc = _get_nc()
    in_maps = make_in_maps(inputs, targets)
    res = run_bass_kernel_spmd(nc, in_maps, list(range(N_CORES)))
    return _reduce_outputs(
        [res.results[c]["res"] for c in range(N_CORES)]
    )


# revision 19
# speedup vs baseline: 1.2130x; 1.0346x over previous
"""Masked-BCE mean loss kernel for Trainium2, data-parallel over 8 NeuronCores.

Math (targets t are exactly 0.0/1.0, C=2 label columns):
    bce(x, t) = softplus(x) - x*t = softplus(y),  y = (1-2t)*x = w*x
    row mask  = 1[t0 + t1 > 0]
    answer    = sum(mask * (bce0 + bce1)) / (B*C)

Work is split across chunks in two modes, balancing DVE against ACT with the
tensor engine absorbing the remainder:

  PAIR chunks (DVE-heavy, ACT-light):
    softplus(y0)+softplus(y1) = ln((1+E0)(1+E1)) = ln(1+r), r = E0+E1+E0*E1.
    The pair mask M' multiplies into r before the Ln (bias=1.0), so masked
    pairs give ln(1)=0 and the Ln's accum_out IS the masked partial sum.
    Ln runs on HALF the elements.

  FULL chunks (ACT-heavy, DVE-light, PE does the masking):
    S = ln(E+1) over all elements; PE accumulates the generalized diagonal
    of M'[128-window]^T x [S0-win | S1-win] (FD=256) into PSUM -- the
    diagonal stripes hold sum(mask * (sp0+sp1)).

Common per-chunk DVE (all unit-stride bf16 -> 2x mode; layout [c0 | c1]
half-blocks per chunk):
    Y  = W * X          (exact: w is +-1)
    V  = min(W0, W1)    (+1 on all-zero-target rows, -1 otherwise)
    M' = -0.5*V + 0.5   (tensor_scalar 2-op, 4x; exact {0,1})

DMA discipline (every dma_start pays ~2us fixed and all transfers FIFO on
one SDMA ring set): X and W ship interleaved in ONE dram tensor, one
dma_start per chunk (large contiguous descriptors), all via nc.sync (HWDGE;
SWDGE would starve behind DVE perf-mode ops).  Outputs merge into one
tensor/one DMA.  Engines run their queues IN ORDER, so emission is
software-pipelined one chunk deep.

Host: answer = (scol part + PSUM diag stripes part) / (B*C) in f64.
"""

import sys

import numpy as np

for _p in ("/opt/trn_rl_repo",):
    if _p not in sys.path:
        sys.path.insert(0, _p)

import concourse.tile as tile  # noqa: E402
from concourse import bacc, mybir  # noqa: E402
from concourse.bass_utils import run_bass_kernel_spmd  # noqa: E402

N_CORES = 8
B = 8388608
C = 2
PAIRS = B // N_CORES  # 1048576 pairs per core
P = 128
COLS = 2 * PAIRS // P  # 16384 data columns per core
HCOLS = COLS // 2  # 8192 pair-columns per core

# chunk schedule: (h pair-cols, mode); FULL chunks first so the PE stripe
# matmuls drain while the PAIR chunks still run
CHUNKS = [
    (512, "full"),
    (1280, "full"),
    (1792, "full"),
    (1024, "full"),
    (1536, "pair"),
    (1536, "pair"),
    (512, "pair"),
]
assert sum(h for h, _ in CHUNKS) == HCOLS
N_CHUNKS = len(CHUNKS)
TOTAL_MM = sum(h // 128 for h, m in CHUNKS if m == "full")

dt = mybir.dt
AF = mybir.ActivationFunctionType
ALU = mybir.AluOpType

_CACHE: dict[str, object] = {}


def _patch_act_tables():
    """Pin Exp and Ln to the single covering table set (one ACT_TABLE_LOAD)."""
    if _CACHE.get("act_patched"):
        return
    import concourse.hw_specs as hw_specs

    orig = hw_specs.get_activation_tables

    def patched(module_arch):
        tabs = orig(module_arch)
        out = {}
        for name, funcs in tabs.items():
            if name == "natural_log_exp_and_others":
                out[name] = set(funcs)
            else:
                out[name] = set(funcs) - {AF.Exp, AF.Ln}
        return out

    bacc.get_activation_tables = patched
    _CACHE["act_patched"] = True


def _build_nc():
    _patch_act_tables()
    nc = bacc.Bacc(
        "TRN2", target_bir_lowering=False, debug=False, num_devices=N_CORES
    )
    # X and W interleaved chunk-major: chunk ci occupies 4*h cols as
    # [X0h | X1h | W0h | W1h]
    xw_d = nc.dram_tensor(
        "xw", [P, 2 * COLS], dt.bfloat16, kind="ExternalInput"
    ).ap()
    # one output tensor: cols 0..255 = PE stripe acc, 256.. = ln accums
    res_d = nc.dram_tensor(
        "res", [P, 256 + N_CHUNKS], dt.float32, kind="ExternalOutput"
    ).ap()

    with tile.TileContext(nc) as tc:
        with (
            tc.tile_pool(name="io", bufs=N_CHUNKS) as io_pool,
            tc.tile_pool(name="work", bufs=2) as work_pool,
            tc.tile_pool(name="ps", bufs=1, space="PSUM") as psum_pool,
            tc.tile_pool(name="outp", bufs=1) as out_pool,
        ):
            # tiny dummy Exp up front hoists the ~2.7us ACT_TABLE_LOAD off
            # the critical path (overlaps the first DMAs)
            warm = out_pool.tile([P, 8], dt.float32)
            nc.vector.memset(warm[:], 0.0)
            nc.scalar.activation(warm[:], warm[:], AF.Exp)

            res = out_pool.tile([P, 256 + N_CHUNKS], dt.float32)
            nc.vector.memset(res[:, 256:], 0.0)
            acc = psum_pool.tile([P, 256], dt.float32)

            # one input DMA per chunk; each chunk owns its tile
            XW = []
            col0 = 0
            for ci, (h, _) in enumerate(CHUNKS):
                T = io_pool.tile([P, 4 * h], dt.bfloat16, tag="XW",
                                 name=f"XW{ci}")
                nc.sync.dma_start(T[:], xw_d[:, col0 : col0 + 4 * h])
                XW.append(T)
                col0 += 4 * h

            E = [None] * N_CHUNKS
            Mp = [None] * N_CHUNKS
            mm_state = [0]

            def stage_B(ci):  # input-side DVE + exp
                h, _ = CHUNKS[ci]
                f = 2 * h
                Xc = XW[ci][:, :f]
                Wc = XW[ci][:, f : 2 * f]
                Y = work_pool.tile([P, f], dt.bfloat16, tag="Y")
                nc.vector.tensor_tensor(Y[:], Wc, Xc, ALU.mult)
                E[ci] = work_pool.tile(
                    [P, f], dt.bfloat16, tag="E", name=f"E{ci}", bufs=3
                )
                nc.scalar.activation(E[ci][:], Y[:], AF.Exp)
                V = work_pool.tile([P, h], dt.bfloat16, tag="V")
                nc.vector.tensor_tensor(
                    V[:], XW[ci][:, f : f + h], XW[ci][:, f + h : 2 * f],
                    ALU.min,
                )
                Mp[ci] = work_pool.tile(
                    [P, h], dt.bfloat16, tag="Mp", name=f"Mp{ci}", bufs=3
                )
                nc.vector.tensor_scalar(
                    Mp[ci][:], V[:], -0.5, 0.5, ALU.mult, ALU.add
                )

            def stage_C(ci):
                h, mode = CHUNKS[ci]
                f = 2 * h
                Ec = E[ci]
                if mode == "pair":
                    P2 = work_pool.tile([P, h], dt.bfloat16, tag="P2")
                    nc.vector.scalar_tensor_tensor(
                        P2[:], Ec[:, :h], 1.0, Ec[:, h:f], ALU.add, ALU.mult
                    )
                    r = work_pool.tile([P, h], dt.bfloat16, tag="r")
                    nc.vector.tensor_tensor(r[:], P2[:], Ec[:, :h], ALU.add)
                    rm = work_pool.tile([P, h], dt.bfloat16, tag="rm")
                    nc.vector.tensor_tensor(rm[:], r[:], Mp[ci][:], ALU.mult)
                    L = work_pool.tile([P, h], dt.bfloat16, tag="L")
                    nc.scalar.activation(
                        L[:], rm[:], AF.Ln, bias=1.0,
                        accum_out=res[:, 256 + ci : 257 + ci],
                    )
                else:
                    S = work_pool.tile([P, f], dt.bfloat16, tag="S")
                    nc.scalar.activation(S[:], Ec[:], AF.Ln, bias=1.0)
                    Sv = S[:].rearrange("p (b h) -> p b h", b=2)
                    for c in range(0, h, 128):
                        nc.tensor.matmul(
                            acc[:],
                            lhsT=Mp[ci][:, c : c + 128],
                            rhs=Sv[:, :, c : c + 128],
                            start=(mm_state[0] == 0),
                            stop=(mm_state[0] == TOTAL_MM - 1),
                        )
                        mm_state[0] += 1

            # software-pipelined emission, two chunks deep
            stage_B(0)
            stage_B(1)
            for ci in range(2, N_CHUNKS):
                stage_B(ci)
                stage_C(ci - 2)
            stage_C(N_CHUNKS - 2)
            stage_C(N_CHUNKS - 1)

            nc.vector.tensor_copy(res[:, :256], acc[:])
            nc.sync.dma_start(res_d[:], res[:])

    nc.compile()
    return nc


def _get_nc():
    if "nc" not in _CACHE:
        _CACHE["nc"] = _build_nc()
    return _CACHE["nc"]


def _reduce_outputs(results: list[np.ndarray]) -> np.ndarray:
    j = np.arange(P)
    total = 0.0
    for re_ in results:
        a64 = re_.astype(np.float64)
        total += a64[:, 256:].sum()  # pair chunks: ln accums
        total += a64[j, j].sum() + a64[j, 128 + j].sum()  # full chunks
    return np.asarray(total / (B * C), dtype=np.float32)


def make_in_maps(inputs: np.ndarray, targets: np.ndarray) -> list[dict]:
    import ml_dtypes

    # Per core, chunk-major interleave [X0h | X1h | W0h | W1h] so one
    # dma_start per chunk carries both tensors; halves are unit-stride
    # blocks so every DVE op runs in 2x mode.  w = 1 - 2t (+-1, exact in
    # bf16) multiplies straight into x on device.
    x = np.ascontiguousarray(inputs, dtype=np.float32).reshape(
        N_CORES, PAIRS, C
    )
    w = 1.0 - 2.0 * np.ascontiguousarray(targets, dtype=np.float32).reshape(
        N_CORES, PAIRS, C
    )
    xp = x.transpose(0, 2, 1).reshape(N_CORES, C, P, HCOLS)
    wp = w.transpose(0, 2, 1).reshape(N_CORES, C, P, HCOLS)

    xw = np.empty((N_CORES, P, 2 * COLS), dtype=ml_dtypes.bfloat16)
    col0 = 0
    off = 0
    for h, _ in CHUNKS:
        xw[:, :, col0 : col0 + h] = xp[:, 0, :, off : off + h]
        xw[:, :, col0 + h : col0 + 2 * h] = xp[:, 1, :, off : off + h]
        xw[:, :, col0 + 2 * h : col0 + 3 * h] = wp[:, 0, :, off : off + h]
        xw[:, :, col0 + 3 * h : col0 + 4 * h] = wp[:, 1, :, off : off + h]
        col0 += 4 * h
        off += h
    return [{"xw": xw[c]} for c in range(N_CORES)]


def kernel(inputs: np.ndarray, targets: np.ndarray) -> np.ndarray:
    nc = _get_nc()
    in_maps = make_in_maps(inputs, targets)
    res = run_bass_kernel_spmd(nc, in_maps, list(range(N_CORES)))
    return _reduce_outputs(
        [res.results[c]["res"] for c in range(N_CORES)]
    )


# revision 24
# speedup vs baseline: 1.2761x; 1.0520x over previous
"""Masked-BCE mean loss kernel for Trainium2, data-parallel over 8 NeuronCores.

Math (targets t are exactly 0.0/1.0, C=2 label columns):
    bce(x, t) = softplus(x) - x*t = softplus(y),  y = (1-2t)*x = w*x
    row mask  = 1[t0 + t1 > 0]
    answer    = sum(mask * (bce0 + bce1)) / (B*C)

Input encoding (halves DMA -- the 8 MiB/core bf16 x+w pair was the HBM/ring
wall): t is stolen into the LSB of bf16 x ("LSB steal").  One 4 MiB tensor
ships per core; x loses its bottom mantissa bit (rel err 2^-8, unbiased
since t is independent of x).  On-device decode is all int16 bitwise ops at
full DVE speed:
    M   = (x' & 1) << 15        (tensor_scalar 2-op, 4x) - sign-flip masks
    Y   = x' ^ M                (tensor_tensor xor, 2x)  - exact w*x
    Vor = M0 | M1               (2x) - 0x8000 iff pair is unmasked
    Mp  = cast(Vor >> 15)       ({0,1} bf16 pair mask)

Work splits across chunks in two modes, balancing DVE against ACT with the
tensor engine absorbing the remainder:

  PAIR chunks (DVE-heavy, ACT-light):
    softplus(y0)+softplus(y1) = ln((1+E0)(1+E1)) = ln(1+r), r = E0+E1+E0*E1.
    Mp multiplies into r before the Ln (bias=1.0), so masked pairs give
    ln(1)=0 and the Ln's accum_out IS the masked partial sum; Ln runs on
    HALF the elements.

  FULL chunks (ACT-heavy, DVE-light, PE masks):
    S = ln(E+1) over all elements; PE accumulates the generalized diagonal
    of Mp[128-window]^T x [S0-win | S1-win] (FD=256) into PSUM -- the
    diagonal stripes hold sum(mask * (sp0+sp1)).

DMA discipline: one dma_start per chunk on nc.sync (HWDGE; every transfer
pays ~2us fixed and all FIFO on one ring set).  Engines run queues IN
ORDER, so emission is software-pipelined two chunks deep.  Exp+Ln pinned to
the single `natural_log_exp_and_others` table set (one ACT_TABLE_LOAD).

Host: answer = (ln accums + PSUM diag stripes) / (B*C) in f64.
"""

import sys

import numpy as np

for _p in ("/opt/trn_rl_repo",):
    if _p not in sys.path:
        sys.path.insert(0, _p)

import concourse.tile as tile  # noqa: E402
from concourse import bacc, mybir  # noqa: E402
from concourse.bass_utils import run_bass_kernel_spmd  # noqa: E402

N_CORES = 8
B = 8388608
C = 2
PAIRS = B // N_CORES  # 1048576 pairs per core
P = 128
COLS = 2 * PAIRS // P  # 16384 data columns per core
HCOLS = COLS // 2  # 8192 pair-columns per core

# chunk schedule: (h pair-cols, mode); FULL chunks first so the PE stripe
# matmuls drain while the PAIR chunks run; small head (fast start) and
# small tail (short serial drain)
CHUNKS = [
    (512, "full"),
    (1536, "full"),
    (1792, "full"),
    (1792, "full"),
    (512, "full"),
    (1536, "pair"),
    (512, "pair"),
]
assert sum(h for h, _ in CHUNKS) == HCOLS
N_CHUNKS = len(CHUNKS)
TOTAL_MM = sum(h // 128 for h, m in CHUNKS if m == "full")

dt = mybir.dt
AF = mybir.ActivationFunctionType
ALU = mybir.AluOpType

_CACHE: dict[str, object] = {}


def _patch_act_tables():
    """Pin Exp and Ln to the single covering table set (one ACT_TABLE_LOAD)."""
    if _CACHE.get("act_patched"):
        return
    import concourse.hw_specs as hw_specs

    orig = hw_specs.get_activation_tables

    def patched(module_arch):
        tabs = orig(module_arch)
        out = {}
        for name, funcs in tabs.items():
            if name == "natural_log_exp_and_others":
                out[name] = set(funcs)
            else:
                out[name] = set(funcs) - {AF.Exp, AF.Ln}
        return out

    bacc.get_activation_tables = patched
    _CACHE["act_patched"] = True


def _build_nc():
    _patch_act_tables()
    nc = bacc.Bacc(
        "TRN2", target_bir_lowering=False, debug=False, num_devices=N_CORES
    )
    xq_d = nc.dram_tensor("xq", [P, COLS], dt.bfloat16, kind="ExternalInput").ap()
    # one output tensor: cols 0..255 = PE stripe acc, 256.. = ln accums
    res_d = nc.dram_tensor(
        "res", [P, 256 + N_CHUNKS], dt.float32, kind="ExternalOutput"
    ).ap()

    with tile.TileContext(nc) as tc:
        with (
            tc.tile_pool(name="io", bufs=N_CHUNKS) as io_pool,
            tc.tile_pool(name="work", bufs=2) as work_pool,
            tc.tile_pool(name="ps", bufs=1, space="PSUM") as psum_pool,
            tc.tile_pool(name="outp", bufs=1) as out_pool,
        ):
            # tiny dummy Exp up front hoists the ~2.7us ACT_TABLE_LOAD off
            # the critical path (overlaps the first DMAs)
            warm = out_pool.tile([P, 8], dt.float32)
            nc.vector.memset(warm[:], 0.0)
            nc.scalar.activation(warm[:], warm[:], AF.Exp)

            res = out_pool.tile([P, 256 + N_CHUNKS], dt.float32)
            nc.vector.memset(res[:, 256:], 0.0)
            acc = psum_pool.tile([P, 256], dt.float32)

            # one input DMA per chunk; each chunk owns its tile
            XQ = []
            col0 = 0
            for ci, (h, _) in enumerate(CHUNKS):
                T = io_pool.tile([P, 2 * h], dt.bfloat16, tag="XQ",
                                 name=f"XQ{ci}")
                nc.sync.dma_start(T[:], xq_d[:, col0 : col0 + 2 * h])
                XQ.append(T)
                col0 += 2 * h

            E = [None] * N_CHUNKS
            Mp = [None] * N_CHUNKS
            mm_state = [0]

            def stage_B(ci):  # decode + exp
                h, _ = CHUNKS[ci]
                f = 2 * h
                Xi = XQ[ci][:].bitcast(dt.uint16)
                M = work_pool.tile([P, f], dt.uint16, tag="M")
                nc.vector.tensor_scalar(
                    M[:], Xi, 1, 15, ALU.bitwise_and, ALU.logical_shift_left
                )
                Yi = work_pool.tile([P, f], dt.uint16, tag="Yi")
                nc.vector.tensor_tensor(Yi[:], Xi, M[:], ALU.bitwise_xor)
                E[ci] = work_pool.tile(
                    [P, f], dt.bfloat16, tag="E", name=f"E{ci}", bufs=3
                )
                nc.scalar.activation(
                    E[ci][:], Yi[:].bitcast(dt.bfloat16), AF.Exp
                )
                Vor = work_pool.tile([P, h], dt.uint16, tag="Vor")
                nc.vector.tensor_tensor(
                    Vor[:], M[:, :h], M[:, h:f], ALU.bitwise_or
                )
                # 0x8000 >> 1 = 0x4000 = bf16 2.0 -> pair mask in {0, 2.0};
                # the factor 2 divides out on the host
                Mpi = work_pool.tile(
                    [P, h], dt.uint16, tag="Mpi", name=f"Mpi{ci}", bufs=3
                )
                nc.vector.tensor_scalar(
                    Mpi[:], Vor[:], 1, None, ALU.logical_shift_right
                )
                Mp[ci] = Mpi[:].bitcast(dt.bfloat16)

            def stage_C(ci):
                h, mode = CHUNKS[ci]
                f = 2 * h
                Ec = E[ci]
                if mode == "pair":
                    P2 = work_pool.tile([P, h], dt.bfloat16, tag="P2")
                    nc.vector.scalar_tensor_tensor(
                        P2[:], Ec[:, :h], 1.0, Ec[:, h:f], ALU.add, ALU.mult
                    )
                    r = work_pool.tile([P, h], dt.bfloat16, tag="r")
                    nc.vector.tensor_tensor(r[:], P2[:], Ec[:, :h], ALU.add)
                    rm = work_pool.tile([P, h], dt.bfloat16, tag="rm")
                    nc.vector.tensor_tensor(rm[:], r[:], Mp[ci], ALU.mult)
                    # Mp is {0, 2.0}; scale=0.5 restores ln(1 + mask*r)
                    L = work_pool.tile([P, h], dt.bfloat16, tag="L")
                    nc.scalar.activation(
                        L[:], rm[:], AF.Ln, bias=1.0, scale=0.5,
                        accum_out=res[:, 256 + ci : 257 + ci],
                    )
                else:
                    S = work_pool.tile([P, f], dt.bfloat16, tag="S")
                    nc.scalar.activation(S[:], Ec[:], AF.Ln, bias=1.0)
                    Sv = S[:].rearrange("p (b h) -> p b h", b=2)
                    for c in range(0, h, 128):
                        nc.tensor.matmul(
                            acc[:],
                            lhsT=Mp[ci][:, c : c + 128],  # {0,2}: /2 on host
                            rhs=Sv[:, :, c : c + 128],
                            start=(mm_state[0] == 0),
                            stop=(mm_state[0] == TOTAL_MM - 1),
                        )
                        mm_state[0] += 1

            # software-pipelined emission, two chunks deep
            stage_B(0)
            stage_B(1)
            for ci in range(2, N_CHUNKS):
                stage_B(ci)
                stage_C(ci - 2)
            stage_C(N_CHUNKS - 2)
            stage_C(N_CHUNKS - 1)

            nc.vector.tensor_copy(res[:, :256], acc[:])
            nc.sync.dma_start(res_d[:], res[:])

    nc.compile()
    return nc


def _get_nc():
    if "nc" not in _CACHE:
        _CACHE["nc"] = _build_nc()
    return _CACHE["nc"]


def _reduce_outputs(results: list[np.ndarray]) -> np.ndarray:
    j = np.arange(P)
    total = 0.0
    for re_ in results:
        a64 = re_.astype(np.float64)
        total += a64[:, 256:].sum()  # pair chunks: ln accums
        # full chunks: stripes carry the {0,2} mask -> halve
        total += 0.5 * (a64[j, j].sum() + a64[j, 128 + j].sum())
    return np.asarray(total / (B * C), dtype=np.float32)


def make_in_maps(inputs: np.ndarray, targets: np.ndarray) -> list[dict]:
    import ml_dtypes

    # Per core, chunk-major [col0-block | col1-block] halves so every DVE op
    # is unit-stride (2x mode); t stolen into the LSB of bf16 x.
    x = np.ascontiguousarray(inputs, dtype=np.float32).reshape(
        N_CORES, PAIRS, C
    )
    t = np.ascontiguousarray(targets, dtype=np.float32).reshape(
        N_CORES, PAIRS, C
    )
    xp = x.transpose(0, 2, 1).reshape(N_CORES, C, P, HCOLS)
    tp = t.transpose(0, 2, 1).reshape(N_CORES, C, P, HCOLS)

    xq = np.empty((N_CORES, P, COLS), dtype=np.uint16)
    col0 = 0
    off = 0
    for h, _ in CHUNKS:
        for c in range(C):
            xb = xp[:, c, :, off : off + h].astype(ml_dtypes.bfloat16)
            tb = tp[:, c, :, off : off + h] != 0.0
            xq[:, :, col0 : col0 + h] = (
                xb.view(np.uint16) & np.uint16(0xFFFE)
            ) | tb.astype(np.uint16)
            col0 += h
        off += h
    return [
        {"xq": xq[c].view(ml_dtypes.bfloat16)} for c in range(N_CORES)
    ]


def kernel(inputs: np.ndarray, targets: np.ndarray) -> np.ndarray:
    nc = _get_nc()
    in_maps = make_in_maps(inputs, targets)
    res = run_bass_kernel_spmd(nc, in_maps, list(range(N_CORES)))
    return _reduce_outputs(
        [res.results[c]["res"] for c in range(N_CORES)]
    )


# revision 26
# speedup vs baseline: 1.3182x; 1.0330x over previous
"""Masked-BCE mean loss kernel for Trainium2, data-parallel over 8 NeuronCores.

Math (targets t are exactly 0.0/1.0, C=2 label columns):
    bce(x, t) = softplus(x) - x*t = softplus(y),  y = (1-2t)*x = w*x
    row mask  = 1[t0 + t1 > 0]
    answer    = sum(mask * (bce0 + bce1)) / (B*C)

Input encoding (halves DMA -- the 8 MiB/core bf16 x+w pair was the HBM/ring
wall): t is stolen into the LSB of bf16 x ("LSB steal").  One 4 MiB tensor
ships per core; x loses its bottom mantissa bit (rel err 2^-8, unbiased
since t is independent of x).  On-device decode is all int16 bitwise ops at
full DVE speed:
    M   = (x' & 1) << 15        (tensor_scalar 2-op, 4x) - sign-flip masks
    Y   = x' ^ M                (tensor_tensor xor, 2x)  - exact w*x
    Vor = M0 | M1               (2x) - 0x8000 iff pair is unmasked
    Mp  = cast(Vor >> 15)       ({0,1} bf16 pair mask)

Work splits across chunks in two modes, balancing DVE against ACT with the
tensor engine absorbing the remainder:

  PAIR chunks (DVE-heavy, ACT-light):
    softplus(y0)+softplus(y1) = ln((1+E0)(1+E1)) = ln(1+r), r = E0+E1+E0*E1.
    Mp multiplies into r before the Ln (bias=1.0), so masked pairs give
    ln(1)=0 and the Ln's accum_out IS the masked partial sum; Ln runs on
    HALF the elements.

  FULL chunks (ACT-heavy, DVE-light, PE masks):
    S = ln(E+1) over all elements; PE accumulates the generalized diagonal
    of Mp[128-window]^T x [S0-win | S1-win] (FD=256) into PSUM -- the
    diagonal stripes hold sum(mask * (sp0+sp1)).

DMA discipline: one dma_start per chunk on nc.sync (HWDGE; every transfer
pays ~2us fixed and all FIFO on one ring set).  Engines run queues IN
ORDER, so emission is software-pipelined two chunks deep.  Exp+Ln pinned to
the single `natural_log_exp_and_others` table set (one ACT_TABLE_LOAD).

Host: answer = (ln accums + PSUM diag stripes) / (B*C) in f64.
"""

import sys

import numpy as np

for _p in ("/opt/trn_rl_repo",):
    if _p not in sys.path:
        sys.path.insert(0, _p)

import concourse.tile as tile  # noqa: E402
from concourse import bacc, mybir  # noqa: E402
from concourse.bass_utils import run_bass_kernel_spmd  # noqa: E402

N_CORES = 8
B = 8388608
C = 2
PAIRS = B // N_CORES  # 1048576 pairs per core
P = 128
COLS = 2 * PAIRS // P  # 16384 data columns per core
HCOLS = COLS // 2  # 8192 pair-columns per core

# chunk schedule: (h pair-cols, mode); FULL chunks first so the PE stripe
# matmuls drain while the PAIR chunks run; small head (fast start) and
# small tail (short serial drain)
CHUNKS = [
    (512, "full"),
    (1792, "full"),
    (1792, "full"),
    (1024, "full"),
    (1536, "pair"),
    (1024, "pair"),
    (512, "pair"),
]
assert sum(h for h, _ in CHUNKS) == HCOLS
N_CHUNKS = len(CHUNKS)
TOTAL_MM = sum(h // 128 for h, _ in CHUNKS)

dt = mybir.dt
AF = mybir.ActivationFunctionType
ALU = mybir.AluOpType

_CACHE: dict[str, object] = {}


def _patch_act_tables():
    """Pin Exp and Ln to the single covering table set (one ACT_TABLE_LOAD)."""
    if _CACHE.get("act_patched"):
        return
    import concourse.hw_specs as hw_specs

    orig = hw_specs.get_activation_tables

    def patched(module_arch):
        tabs = orig(module_arch)
        out = {}
        for name, funcs in tabs.items():
            if name == "natural_log_exp_and_others":
                out[name] = set(funcs)
            else:
                out[name] = set(funcs) - {AF.Exp, AF.Ln}
        return out

    bacc.get_activation_tables = patched
    _CACHE["act_patched"] = True


def _build_nc():
    _patch_act_tables()
    nc = bacc.Bacc(
        "TRN2", target_bir_lowering=False, debug=False, num_devices=N_CORES
    )
    xq_d = nc.dram_tensor("xq", [P, COLS], dt.bfloat16, kind="ExternalInput").ap()
    # one output tensor: cols 0..255 = PE stripe acc, 256.. = ln accums
    res_d = nc.dram_tensor(
        "res", [P, 256 + N_CHUNKS], dt.float32, kind="ExternalOutput"
    ).ap()

    with tile.TileContext(nc) as tc:
        with (
            tc.tile_pool(name="io", bufs=N_CHUNKS) as io_pool,
            tc.tile_pool(name="work", bufs=2) as work_pool,
            tc.tile_pool(name="ps", bufs=1, space="PSUM") as psum_pool,
            tc.tile_pool(name="outp", bufs=1) as out_pool,
        ):
            # tiny dummy Exp up front hoists the ~2.7us ACT_TABLE_LOAD off
            # the critical path (overlaps the first DMAs)
            warm = out_pool.tile([P, 8], dt.float32)
            nc.vector.memset(warm[:], 0.0)
            nc.scalar.activation(warm[:], warm[:], AF.Exp)

            res = out_pool.tile([P, 256 + N_CHUNKS], dt.float32)
            nc.vector.memset(res[:, 256:], 0.0)
            acc = psum_pool.tile([P, 256], dt.float32)

            # one input DMA per chunk; each chunk owns its tile
            XQ = []
            col0 = 0
            for ci, (h, _) in enumerate(CHUNKS):
                T = io_pool.tile([P, 2 * h], dt.bfloat16, tag="XQ",
                                 name=f"XQ{ci}")
                nc.sync.dma_start(T[:], xq_d[:, col0 : col0 + 2 * h])
                XQ.append(T)
                col0 += 2 * h

            E = [None] * N_CHUNKS
            Mp = [None] * N_CHUNKS
            mm_state = [0]

            def stage_B(ci):  # decode + exp
                h, _ = CHUNKS[ci]
                f = 2 * h
                Xi = XQ[ci][:].bitcast(dt.uint16)
                M = work_pool.tile([P, f], dt.uint16, tag="M")
                nc.vector.tensor_scalar(
                    M[:], Xi, 1, 15, ALU.bitwise_and, ALU.logical_shift_left
                )
                Yi = work_pool.tile([P, f], dt.uint16, tag="Yi")
                nc.vector.tensor_tensor(Yi[:], Xi, M[:], ALU.bitwise_xor)
                E[ci] = work_pool.tile(
                    [P, f], dt.bfloat16, tag="E", name=f"E{ci}", bufs=3
                )
                nc.scalar.activation(
                    E[ci][:], Yi[:].bitcast(dt.bfloat16), AF.Exp
                )
                Vor = work_pool.tile([P, h], dt.uint16, tag="Vor")
                nc.vector.tensor_tensor(
                    Vor[:], M[:, :h], M[:, h:f], ALU.bitwise_or
                )
                # 0x8000 >> 1 = 0x4000 = bf16 2.0 -> pair mask in {0, 2.0};
                # the factor 2 divides out on the host
                Mpi = work_pool.tile(
                    [P, h], dt.uint16, tag="Mpi", name=f"Mpi{ci}", bufs=3
                )
                nc.vector.tensor_scalar(
                    Mpi[:], Vor[:], 1, None, ALU.logical_shift_right
                )
                Mp[ci] = Mpi[:].bitcast(dt.bfloat16)

            def stage_C(ci):
                h, mode = CHUNKS[ci]
                f = 2 * h
                Ec = E[ci]
                if mode == "pair":
                    P2 = work_pool.tile([P, h], dt.bfloat16, tag="P2")
                    nc.vector.scalar_tensor_tensor(
                        P2[:], Ec[:, :h], 1.0, Ec[:, h:f], ALU.add, ALU.mult
                    )
                    r = work_pool.tile([P, h], dt.bfloat16, tag="r")
                    nc.vector.tensor_tensor(r[:], P2[:], Ec[:, :h], ALU.add)
                    # unmasked pair softplus sums; PE applies the mask
                    L = work_pool.tile([P, h], dt.bfloat16, tag="L")
                    nc.scalar.activation(L[:], r[:], AF.Ln, bias=1.0)
                    for c in range(0, h, 128):
                        nc.tensor.matmul(
                            acc[:, :128],
                            lhsT=Mp[ci][:, c : c + 128],  # {0,2}: /2 on host
                            rhs=L[:, c : c + 128],
                            start=(mm_state[0] == 0),
                            stop=(mm_state[0] == TOTAL_MM - 1),
                        )
                        mm_state[0] += 1
                else:
                    S = work_pool.tile([P, f], dt.bfloat16, tag="S")
                    nc.scalar.activation(S[:], Ec[:], AF.Ln, bias=1.0)
                    Sv = S[:].rearrange("p (b h) -> p b h", b=2)
                    for c in range(0, h, 128):
                        nc.tensor.matmul(
                            acc[:],
                            lhsT=Mp[ci][:, c : c + 128],  # {0,2}: /2 on host
                            rhs=Sv[:, :, c : c + 128],
                            start=(mm_state[0] == 0),
                            stop=(mm_state[0] == TOTAL_MM - 1),
                        )
                        mm_state[0] += 1

            # software-pipelined emission, two chunks deep
            stage_B(0)
            stage_B(1)
            for ci in range(2, N_CHUNKS):
                stage_B(ci)
                stage_C(ci - 2)
            stage_C(N_CHUNKS - 2)
            stage_C(N_CHUNKS - 1)

            nc.vector.tensor_copy(res[:, :256], acc[:])
            nc.sync.dma_start(res_d[:], res[:])

    nc.compile()
    return nc


def _get_nc():
    if "nc" not in _CACHE:
        _CACHE["nc"] = _build_nc()
    return _CACHE["nc"]


def _reduce_outputs(results: list[np.ndarray]) -> np.ndarray:
    j = np.arange(P)
    total = 0.0
    for re_ in results:
        a64 = re_.astype(np.float64)
        total += a64[:, 256:].sum()  # pair chunks: ln accums
        # full chunks: stripes carry the {0,2} mask -> halve
        total += 0.5 * (a64[j, j].sum() + a64[j, 128 + j].sum())
    return np.asarray(total / (B * C), dtype=np.float32)


def make_in_maps(inputs: np.ndarray, targets: np.ndarray) -> list[dict]:
    import ml_dtypes

    # Per core, chunk-major [col0-block | col1-block] halves so every DVE op
    # is unit-stride (2x mode); t stolen into the LSB of bf16 x.
    x = np.ascontiguousarray(inputs, dtype=np.float32).reshape(
        N_CORES, PAIRS, C
    )
    t = np.ascontiguousarray(targets, dtype=np.float32).reshape(
        N_CORES, PAIRS, C
    )
    xp = x.transpose(0, 2, 1).reshape(N_CORES, C, P, HCOLS)
    tp = t.transpose(0, 2, 1).reshape(N_CORES, C, P, HCOLS)

    xq = np.empty((N_CORES, P, COLS), dtype=np.uint16)
    col0 = 0
    off = 0
    for h, _ in CHUNKS:
        for c in range(C):
            xb = xp[:, c, :, off : off + h].astype(ml_dtypes.bfloat16)
            tb = tp[:, c, :, off : off + h] != 0.0
            xq[:, :, col0 : col0 + h] = (
                xb.view(np.uint16) & np.uint16(0xFFFE)
            ) | tb.astype(np.uint16)
            col0 += h
        off += h
    return [
        {"xq": xq[c].view(ml_dtypes.bfloat16)} for c in range(N_CORES)
    ]


def kernel(inputs: np.ndarray, targets: np.ndarray) -> np.ndarray:
    nc = _get_nc()
    in_maps = make_in_maps(inputs, targets)
    res = run_bass_kernel_spmd(nc, in_maps, list(range(N_CORES)))
    return _reduce_outputs(
        [res.results[c]["res"] for c in range(N_CORES)]
    )


# revision 28
# speedup vs baseline: 1.3363x; 1.0137x over previous
"""Masked-BCE mean loss kernel for Trainium2, data-parallel over 8 NeuronCores.

Math (targets t are exactly 0.0/1.0, C=2 label columns):
    bce(x, t) = softplus(x) - x*t = softplus(y),  y = (1-2t)*x = w*x
    row mask  = 1[t0 + t1 > 0]
    answer    = sum(mask * (bce0 + bce1)) / (B*C)

Input encoding (halves DMA -- the 8 MiB/core bf16 x+w pair was the HBM/ring
wall): t is stolen into the LSB of bf16 x ("LSB steal").  One 4 MiB tensor
ships per core; x loses its bottom mantissa bit (rel err 2^-8, unbiased
since t is independent of x).  On-device decode is all int16 bitwise ops at
full DVE speed:
    M   = (x' & 1) << 15        (tensor_scalar 2-op, 4x) - sign-flip masks
    Y   = x' ^ M                (tensor_tensor xor, 2x)  - exact w*x
    Vor = M0 | M1               (2x) - 0x8000 iff pair is unmasked
    Mp  = cast(Vor >> 15)       ({0,1} bf16 pair mask)

Work splits across chunks in two modes, balancing DVE against ACT with the
tensor engine absorbing the remainder:

  PAIR chunks (DVE-heavy, ACT-light):
    softplus(y0)+softplus(y1) = ln((1+E0)(1+E1)) = ln(1+r), r = E0+E1+E0*E1.
    Mp multiplies into r before the Ln (bias=1.0), so masked pairs give
    ln(1)=0 and the Ln's accum_out IS the masked partial sum; Ln runs on
    HALF the elements.

  FULL chunks (ACT-heavy, DVE-light, PE masks):
    S = ln(E+1) over all elements; PE accumulates the generalized diagonal
    of Mp[128-window]^T x [S0-win | S1-win] (FD=256) into PSUM -- the
    diagonal stripes hold sum(mask * (sp0+sp1)).

DMA discipline: one dma_start per chunk on nc.sync (HWDGE; every transfer
pays ~2us fixed and all FIFO on one ring set).  Engines run queues IN
ORDER, so emission is software-pipelined two chunks deep.  Exp+Ln pinned to
the single `natural_log_exp_and_others` table set (one ACT_TABLE_LOAD).

Host: answer = (ln accums + PSUM diag stripes) / (B*C) in f64.
"""

import sys

import numpy as np

for _p in ("/opt/trn_rl_repo",):
    if _p not in sys.path:
        sys.path.insert(0, _p)

import concourse.tile as tile  # noqa: E402
from concourse import bacc, mybir  # noqa: E402
from concourse.bass_utils import run_bass_kernel_spmd  # noqa: E402

N_CORES = 8
B = 8388608
C = 2
PAIRS = B // N_CORES  # 1048576 pairs per core
P = 128
COLS = 2 * PAIRS // P  # 16384 data columns per core
HCOLS = COLS // 2  # 8192 pair-columns per core

# chunk schedule: (h pair-cols, mode); FULL chunks first so the PE stripe
# matmuls drain while the PAIR chunks run; small head (fast start) and
# small tail (short serial drain)
CHUNKS = [
    (512, "full"),
    (1792, "full"),
    (1792, "full"),
    (1792, "pair"),
    (1792, "pair"),
    (512, "pairacc"),
]
assert sum(h for h, _ in CHUNKS) == HCOLS
N_CHUNKS = len(CHUNKS)
TOTAL_MM = sum(h // 128 for h, m in CHUNKS if m != "pairacc")

dt = mybir.dt
AF = mybir.ActivationFunctionType
ALU = mybir.AluOpType

_CACHE: dict[str, object] = {}


def _patch_act_tables():
    """Pin Exp and Ln to the single covering table set (one ACT_TABLE_LOAD)."""
    if _CACHE.get("act_patched"):
        return
    import concourse.hw_specs as hw_specs

    orig = hw_specs.get_activation_tables

    def patched(module_arch):
        tabs = orig(module_arch)
        out = {}
        for name, funcs in tabs.items():
            if name == "natural_log_exp_and_others":
                out[name] = set(funcs)
            else:
                out[name] = set(funcs) - {AF.Exp, AF.Ln}
        return out

    bacc.get_activation_tables = patched
    _CACHE["act_patched"] = True


def _build_nc():
    _patch_act_tables()
    nc = bacc.Bacc(
        "TRN2", target_bir_lowering=False, debug=False, num_devices=N_CORES
    )
    xq_d = nc.dram_tensor("xq", [P, COLS], dt.bfloat16, kind="ExternalInput").ap()
    # one output tensor: cols 0..255 = PE stripe acc, 256.. = ln accums
    res_d = nc.dram_tensor(
        "res", [P, 256 + N_CHUNKS], dt.float32, kind="ExternalOutput"
    ).ap()

    with tile.TileContext(nc) as tc:
        with (
            tc.tile_pool(name="io", bufs=N_CHUNKS) as io_pool,
            tc.tile_pool(name="work", bufs=2) as work_pool,
            tc.tile_pool(name="ps", bufs=1, space="PSUM") as psum_pool,
            tc.tile_pool(name="outp", bufs=1) as out_pool,
        ):
            # tiny dummy Exp up front hoists the ~2.7us ACT_TABLE_LOAD off
            # the critical path (overlaps the first DMAs)
            warm = out_pool.tile([P, 8], dt.float32)
            nc.vector.memset(warm[:], 0.0)
            nc.scalar.activation(warm[:], warm[:], AF.Exp)

            res = out_pool.tile([P, 256 + N_CHUNKS], dt.float32)
            nc.vector.memset(res[:, 256:], 0.0)
            acc = psum_pool.tile([P, 256], dt.float32)

            # one input DMA per chunk; each chunk owns its tile
            XQ = []
            col0 = 0
            for ci, (h, _) in enumerate(CHUNKS):
                T = io_pool.tile([P, 2 * h], dt.bfloat16, tag="XQ",
                                 name=f"XQ{ci}")
                nc.sync.dma_start(T[:], xq_d[:, col0 : col0 + 2 * h])
                XQ.append(T)
                col0 += 2 * h

            E = [None] * N_CHUNKS
            Mp = [None] * N_CHUNKS
            mm_state = [0]

            def stage_B(ci):  # decode + exp
                h, _ = CHUNKS[ci]
                f = 2 * h
                Xi = XQ[ci][:].bitcast(dt.uint16)
                M = work_pool.tile([P, f], dt.uint16, tag="M")
                nc.vector.tensor_scalar(
                    M[:], Xi, 1, 15, ALU.bitwise_and, ALU.logical_shift_left
                )
                Yi = work_pool.tile([P, f], dt.uint16, tag="Yi")
                nc.vector.tensor_tensor(Yi[:], Xi, M[:], ALU.bitwise_xor)
                E[ci] = work_pool.tile(
                    [P, f], dt.bfloat16, tag="E", name=f"E{ci}", bufs=3
                )
                nc.scalar.activation(
                    E[ci][:], Yi[:].bitcast(dt.bfloat16), AF.Exp
                )
                Vor = work_pool.tile([P, h], dt.uint16, tag="Vor")
                nc.vector.tensor_tensor(
                    Vor[:], M[:, :h], M[:, h:f], ALU.bitwise_or
                )
                # 0x8000 >> 1 = 0x4000 = bf16 2.0 -> pair mask in {0, 2.0};
                # the factor 2 divides out on the host
                Mpi = work_pool.tile(
                    [P, h], dt.uint16, tag="Mpi", name=f"Mpi{ci}", bufs=3
                )
                nc.vector.tensor_scalar(
                    Mpi[:], Vor[:], 1, None, ALU.logical_shift_right
                )
                Mp[ci] = Mpi[:].bitcast(dt.bfloat16)

            def stage_C(ci):
                h, mode = CHUNKS[ci]
                f = 2 * h
                Ec = E[ci]
                if mode.startswith("pair"):
                    P2 = work_pool.tile([P, h], dt.bfloat16, tag="P2")
                    nc.vector.scalar_tensor_tensor(
                        P2[:], Ec[:, :h], 1.0, Ec[:, h:f], ALU.add, ALU.mult
                    )
                    r = work_pool.tile([P, h], dt.bfloat16, tag="r")
                    nc.vector.tensor_tensor(r[:], P2[:], Ec[:, :h], ALU.add)
                    if mode == "pairacc":
                        # mask on DVE + fused ln accum -> no trailing matmuls
                        # (used for the last chunk: short drain).  Mp is
                        # {0, 2.0}; scale=0.5 restores ln(1 + mask*r).
                        rm = work_pool.tile([P, h], dt.bfloat16, tag="rm")
                        nc.vector.tensor_tensor(
                            rm[:], r[:], Mp[ci], ALU.mult
                        )
                        L = work_pool.tile([P, h], dt.bfloat16, tag="L")
                        nc.scalar.activation(
                            L[:], rm[:], AF.Ln, bias=1.0, scale=0.5,
                            accum_out=res[:, 256 + ci : 257 + ci],
                        )
                        return
                    # unmasked pair softplus sums; PE applies the mask
                    L = work_pool.tile([P, h], dt.bfloat16, tag="L")
                    nc.scalar.activation(L[:], r[:], AF.Ln, bias=1.0)
                    for c in range(0, h, 128):
                        nc.tensor.matmul(
                            acc[:, :128],
                            lhsT=Mp[ci][:, c : c + 128],  # {0,2}: /2 on host
                            rhs=L[:, c : c + 128],
                            start=(mm_state[0] == 0),
                            stop=(mm_state[0] == TOTAL_MM - 1),
                        )
                        mm_state[0] += 1
                else:
                    S = work_pool.tile([P, f], dt.bfloat16, tag="S")
                    nc.scalar.activation(S[:], Ec[:], AF.Ln, bias=1.0)
                    Sv = S[:].rearrange("p (b h) -> p b h", b=2)
                    for c in range(0, h, 128):
                        nc.tensor.matmul(
                            acc[:],
                            lhsT=Mp[ci][:, c : c + 128],  # {0,2}: /2 on host
                            rhs=Sv[:, :, c : c + 128],
                            start=(mm_state[0] == 0),
                            stop=(mm_state[0] == TOTAL_MM - 1),
                        )
                        mm_state[0] += 1

            # software-pipelined emission, two chunks deep
            stage_B(0)
            stage_B(1)
            for ci in range(2, N_CHUNKS):
                stage_B(ci)
                stage_C(ci - 2)
            stage_C(N_CHUNKS - 2)
            stage_C(N_CHUNKS - 1)

            nc.vector.tensor_copy(res[:, :256], acc[:])
            nc.sync.dma_start(res_d[:], res[:])

    nc.compile()
    return nc


def _get_nc():
    if "nc" not in _CACHE:
        _CACHE["nc"] = _build_nc()
    return _CACHE["nc"]


def _reduce_outputs(results: list[np.ndarray]) -> np.ndarray:
    j = np.arange(P)
    total = 0.0
    for re_ in results:
        a64 = re_.astype(np.float64)
        total += a64[:, 256:].sum()  # pair chunks: ln accums
        # full chunks: stripes carry the {0,2} mask -> halve
        total += 0.5 * (a64[j, j].sum() + a64[j, 128 + j].sum())
    return np.asarray(total / (B * C), dtype=np.float32)


def make_in_maps(inputs: np.ndarray, targets: np.ndarray) -> list[dict]:
    import ml_dtypes

    # Per core, chunk-major [col0-block | col1-block] halves so every DVE op
    # is unit-stride (2x mode); t stolen into the LSB of bf16 x.
    x = np.ascontiguousarray(inputs, dtype=np.float32).reshape(
        N_CORES, PAIRS, C
    )
    t = np.ascontiguousarray(targets, dtype=np.float32).reshape(
        N_CORES, PAIRS, C
    )
    xp = x.transpose(0, 2, 1).reshape(N_CORES, C, P, HCOLS)
    tp = t.transpose(0, 2, 1).reshape(N_CORES, C, P, HCOLS)

    xq = np.empty((N_CORES, P, COLS), dtype=np.uint16)
    col0 = 0
    off = 0
    for h, _ in CHUNKS:
        for c in range(C):
            xb = xp[:, c, :, off : off + h].astype(ml_dtypes.bfloat16)
            tb = tp[:, c, :, off : off + h] != 0.0
            xq[:, :, col0 : col0 + h] = (
                xb.view(np.uint16) & np.uint16(0xFFFE)
            ) | tb.astype(np.uint16)
            col0 += h
        off += h
    return [
        {"xq": xq[c].view(ml_dtypes.bfloat16)} for c in range(N_CORES)
    ]


def kernel(inputs: np.ndarray, targets: np.ndarray) -> np.ndarray:
    nc = _get_nc()
    in_maps = make_in_maps(inputs, targets)
    res = run_bass_kernel_spmd(nc, in_maps, list(range(N_CORES)))
    return _reduce_outputs(
        [res.results[c]["res"] for c in range(N_CORES)]
    )
